# revision 16
# baseline (speedup 1.0000x reference)
"""Trainium2 Bass kernel for nn_CMDPEncoder (VQ codebook quantize + random
batch-mix dequantize + DP noise).

Reference semantics:
    dots = einsum('bsd,vd->bsv', base, codebook)
    qi   = argmin_v(csq[v] - 2*dots)                  # [B,S]
    codes[b,s,j] = qi[rand_idx[b,s,j], s]
    out  = mean_j codebook[codes] + 0.1*noise

Sharding: split the sequence dim S across the 8 cores (64 positions each).
The rand_idx mixing crosses only the batch dim at fixed s, so with S-sharding
every core's mixing is fully local (no collectives).  Tokens are laid out
s-major (t = s_local*16 + b) so each 128-token tile holds 8 complete
s-groups of 16 batches, and the mix becomes a block-diagonal [128,128]
matmul with host-precomputed weights (counts/4 from rand_idx).

v4 design:
  - scores accumulate in [128,1024] (2-bank) PSUM tiles; csq bias comes in
    via ACT-engine PSUM prefill (fp32 exact) + all-start=False matmuls.
    A start=False matmul on a never-opened bank does not accumulate
    correctly on HW (v2 post-mortem), so 6 throwaway start=True matmuls
    open all 6 score banks at kernel start.
  - prefills are issued 2 blocks ahead so the in-order ACT engine finishes
    them under the previous blocks' matmuls instead of stalling the PE.
  - tiles 0 and 1 are interleaved per v-pair at the head so the PE does two
    tiles of work per codebook v-block DMA arrival (the 6.3 MB cbT stream
    can't otherwise keep up with one tile's consumption rate).
  - tiles 0-5 scan with one full MAX8 + FIND_INDEX8; tiles 6-7 scan
    per-v-pair incrementally (overlapped under their own scoring) so the
    serial tail after the last matmul is short; the last tile merges
    local FIND indices via an iota-select instead of a full-array FIND.
  - the exact top-2 rescore is ONE difference-dot: GpSimd computes
    g1 - g0 (inc. the csq column), DVE does a single accumulate-dot and a
    sign test.  Dot ops for tile t are emitted one tile late so the
    in-order DVE queue never head-of-line blocks on the GpSimd gathers.
  - mix matmul in bf16 (w is exact in bf16; y gathered from a bf16 copy of
    the codebook).  All noise adds via ACT-drain + SWDGE accumulate-DMA
    (keeps DVE out of the pipeline tail).
"""

import os
import sys

for p in ("/opt/trn_rl_repo",):
    if p not in sys.path:
        sys.path.insert(0, p)

import numpy as np

import concourse.bacc as bacc
import concourse.bass as bass
import concourse.mybir as mybir
import concourse.tile as tile
from concourse.bass_utils import run_bass_kernel_spmd

B, S, D, V, K = 16, 512, 768, 4096, 4
N_CORES = 8
SS = S // N_CORES            # 64 sequence positions per core
T = SS * B                   # 1024 tokens per core, t = s_local*16 + b
TT = T // 128                # 8 token tiles per core
KC = D // 128                # 6 contraction chunks
NV = V // 512                # 8 V-tiles
NVP = V // 1024              # 4 V-pairs (2-bank PSUM tiles)
DP_EPSILON = 0.1
CSQ_CENTER = 768.0
DE = 776                     # padded cb_ext row: 768 cb + 1 csq + 7 pad

F32 = mybir.dt.float32
F16 = mybir.dt.float16
BF16 = mybir.dt.bfloat16
U32 = mybir.dt.uint32
I32 = mybir.dt.int32

_CACHED = {}


def _build_nc():
    nc = bacc.Bacc("TRN2", target_bir_lowering=False, debug=False,
                   num_devices=N_CORES)

    xT_d = nc.dram_tensor("xT", [128, KC * T], BF16, kind="ExternalInput")
    cbT_d = nc.dram_tensor("cbT", [128, KC * V], BF16, kind="ExternalInput")
    csqbc_d = nc.dram_tensor("csqbc", [128, V], F32, kind="ExternalInput")
    csqL_d = nc.dram_tensor("csqL", [2, 128], F16, kind="ExternalInput")
    csqR_d = nc.dram_tensor("csqR", [2, V], F16, kind="ExternalInput")
    cbe_d = nc.dram_tensor("cbe", [V, DE], F32, kind="ExternalInput")
    cbyb_d = nc.dram_tensor("cbyb", [V, D], BF16, kind="ExternalInput")
    w_d = nc.dram_tensor("w", [128, TT * 128], BF16, kind="ExternalInput")
    noise_d = nc.dram_tensor("noise", [T, D], F32, kind="ExternalInput")
    xn_d = nc.dram_tensor("xn", [128, TT * D], F32, kind="ExternalInput")
    mc_d = nc.dram_tensor("mc", [128, 64], F32, kind="ExternalInput")
    out_d = nc.dram_tensor("out", [T, D], F32, kind="ExternalOutput")

    with tile.TileContext(nc) as tc:
        with (
            tc.tile_pool(name="big", bufs=1) as big,
            tc.tile_pool(name="work", bufs=2) as work,
            tc.tile_pool(name="sc", bufs=3) as sc_pool,
            tc.tile_pool(name="ypool", bufs=4) as ypool,
            tc.tile_pool(name="io", bufs=3) as io,
            tc.tile_pool(name="ps_s", bufs=3, space="PSUM") as ps_s,
            tc.tile_pool(name="ps_m", bufs=1, space="PSUM") as ps_m,
        ):
            XTW = KC * 128   # xt columns per token tile
            VBW = KC * 512   # codebook columns per v-block
            # --- input staging, ordered so the PE can start ASAP and the
            # cbT stream stays ahead of the (tile0,tile1)-interleaved head ---
            csql = big.tile([2, 128], F16)
            csqr = big.tile([2, V], F16)
            nc.sync.dma_start(csql[:], csqL_d.ap())
            nc.sync.dma_start(csqr[:], csqR_d.ap())
            csqbc = big.tile([128, V], F32)
            nc.sync.dma_start(csqbc[:, 0:1024], csqbc_d.ap()[:, 0:1024])
            xt_t = [None] * TT
            for t in (0, 1):
                tl = big.tile([128, XTW], BF16, tag=f"xt{t}")
                nc.sync.dma_start(tl[:, 0:3 * 128], xT_d.ap()[:, t * XTW:t * XTW + 3 * 128])
                nc.sync.dma_start(tl[:, 3 * 128:XTW], xT_d.ap()[:, t * XTW + 3 * 128:(t + 1) * XTW])
                xt_t[t] = tl
            cb_t = []
            tl = big.tile([128, VBW], BF16, tag="cbv0")
            for k in range(KC):
                nc.sync.dma_start(tl[:, k * 512:(k + 1) * 512],
                                  cbT_d.ap()[:, k * 512:(k + 1) * 512])
            cb_t.append(tl)
            tl = big.tile([128, VBW], BF16, tag="cbv1")
            nc.sync.dma_start(tl[:], cbT_d.ap()[:, VBW:2 * VBW])
            cb_t.append(tl)
            nc.sync.dma_start(csqbc[:, 1024:2048], csqbc_d.ap()[:, 1024:2048])
            for v in (2, 3):
                tl = big.tile([128, VBW], BF16, tag=f"cbv{v}")
                nc.sync.dma_start(tl[:], cbT_d.ap()[:, v * VBW:(v + 1) * VBW])
                cb_t.append(tl)
            nc.sync.dma_start(csqbc[:, 2048:3072], csqbc_d.ap()[:, 2048:3072])
            for v in (4, 5):
                tl = big.tile([128, VBW], BF16, tag=f"cbv{v}")
                nc.sync.dma_start(tl[:], cbT_d.ap()[:, v * VBW:(v + 1) * VBW])
                cb_t.append(tl)
            nc.sync.dma_start(csqbc[:, 3072:4096], csqbc_d.ap()[:, 3072:4096])
            for v in (6, 7):
                tl = big.tile([128, VBW], BF16, tag=f"cbv{v}")
                nc.sync.dma_start(tl[:], cbT_d.ap()[:, v * VBW:(v + 1) * VBW])
                cb_t.append(tl)
            xn_t = [None] * TT
            for t in (0, 1):
                tl = big.tile([128, D], F32, tag=f"xn{t}")
                nc.sync.dma_start(tl[:], xn_d.ap()[:, t * D:(t + 1) * D])
                xn_t[t] = tl
            for t in range(2, TT):
                tl = big.tile([128, XTW], BF16, tag=f"xt{t}")
                nc.sync.dma_start(tl[:], xT_d.ap()[:, t * XTW:(t + 1) * XTW])
                xt_t[t] = tl
                tl = big.tile([128, D], F32, tag=f"xn{t}")
                nc.sync.dma_start(tl[:], xn_d.ap()[:, t * D:(t + 1) * D])
                xn_t[t] = tl
            w = big.tile([128, TT * 128], BF16)
            nc.sync.dma_start(w[:], w_d.ap())
            # merge constants for the last tile: cols 0:32 iota, 32:64
            # block offsets (1024*(c//8))
            mconst = big.tile([128, 64], F32)
            nc.sync.dma_start(mconst[:], mc_d.ap())

            # block schedule: tiles 0/1 interleaved pairwise at the head so
            # the PE does two tiles of work per codebook v-block arrival,
            # then tiles 2..7 sequential.
            sched = [(0, 0), (1, 0), (0, 1), (1, 1), (0, 2), (1, 2),
                     (0, 3), (1, 3)]
            for t in range(2, TT):
                sched += [(t, vp) for vp in range(NVP)]

            # open all 6 score PSUM banks with throwaway start=True matmuls
            # (values are overwritten by the first prefill of each buffer)
            for _ in range(3):
                psd = ps_s.tile([128, 1024], F32, tag="ps_score")
                for h in range(2):
                    nc.tensor.matmul(psd[:, h * 512:(h + 1) * 512],
                                     csql[:], csqr[:, 0:512],
                                     start=True, stop=True)

            ps_of = {}

            def emit_prefill(j):
                """ACT-prefill the csq bias for schedule slot j."""
                _, vp = sched[j]
                ps = ps_s.tile([128, 1024], F32, tag="ps_score")
                nc.scalar.copy(out=ps[:],
                               in_=csqbc[:, vp * 1024:(vp + 1) * 1024])
                ps_of[j] = ps

            emit_prefill(0)
            emit_prefill(1)

            def emit_block(j, scores):
                """12 bf16 matmuls for schedule slot j, drain to SBUF."""
                t, vp = sched[j]
                vsl = slice(vp * 1024, (vp + 1) * 1024)
                ps = ps_of.pop(j)
                for h in range(2):
                    v = vp * 2 + h
                    hs = slice(h * 512, (h + 1) * 512)
                    for k in range(KC):
                        nc.tensor.matmul(
                            ps[:, hs],
                            xt_t[t][:, k * 128:(k + 1) * 128],
                            cb_t[v][:, k * 512:(k + 1) * 512],
                            start=False, stop=(k == KC - 1),
                            skip_group_check=True)
                if j + 2 < len(sched):
                    emit_prefill(j + 2)
                nc.scalar.copy(out=scores[:, vsl], in_=ps[:])

            def emit_cands(scores, vals32, idxl):
                """global top-2 candidate indices [128,1] i32 each."""
                mv8 = work.tile([128, 8], F32, tag="mv8")
                if vals32 is None:
                    nc.vector.max(mv8[:], scores[:])
                else:
                    nc.vector.max(mv8[:], vals32[:])
                if idxl is None:
                    idx = work.tile([128, 8], U32, tag="idx")
                    nc.vector.max_index(idx[:], mv8[:], scores[:])
                    cand = []
                    for jj in range(2):
                        cj = work.tile([128, 1], I32, tag=f"cand{jj}")
                        nc.vector.tensor_copy(cj[:], idx[:, jj:jj + 1])
                        cand.append(cj)
                    return cand
                # last tile: merge the 4 per-block top-8 indices instead of
                # a full-array FIND (shorter serial tail)
                idxgf = work.tile([128, 32], F32, tag="idxgf")
                nc.vector.tensor_copy(idxgf[:], idxl[:])
                nc.vector.tensor_tensor(out=idxgf[:], in0=idxgf[:],
                                        in1=mconst[:, 32:64],
                                        op=mybir.AluOpType.add)
                pos8 = work.tile([128, 8], U32, tag="pos8")
                nc.vector.max_index(pos8[:], mv8[:], vals32[:])
                cand = []
                for jj in range(2):
                    posf = work.tile([128, 1], F32, tag=f"posf{jj}")
                    nc.vector.tensor_copy(posf[:], pos8[:, jj:jj + 1])
                    m = work.tile([128, 32], F32, tag=f"m{jj}")
                    nc.vector.tensor_scalar(
                        out=m[:], in0=mconst[:, 0:32], scalar1=posf[:, :1],
                        scalar2=None, op0=mybir.AluOpType.is_equal)
                    junk = work.tile([128, 32], F32, tag=f"junk{jj}")
                    candf = work.tile([128, 1], F32, tag=f"candf{jj}")
                    nc.vector.scalar_tensor_tensor(
                        out=junk[:], in0=m[:], scalar=1.0, in1=idxgf[:],
                        op0=mybir.AluOpType.bypass,
                        op1=mybir.AluOpType.mult, accum_out=candf[:])
                    cj = work.tile([128, 1], I32, tag=f"cand{jj}")
                    nc.vector.tensor_copy(cj[:], candf[:])
                    cand.append(cj)
                return cand

            def fixup_issue(t, cand):
                """gather the two candidate rows, compute g1-g0 on GpSimd."""
                gs = []
                for jj in range(2):
                    g = work.tile([128, DE], F32, tag=f"g{jj}",
                                  name=f"g{jj}_{t}")
                    nc.gpsimd.indirect_dma_start(
                        out=g[:], out_offset=None, in_=cbe_d.ap(),
                        in_offset=bass.IndirectOffsetOnAxis(
                            ap=cand[jj][:, :1], axis=0))
                    gs.append(g)
                gdiff = work.tile([128, DE], F32, tag="gdiff",
                                  name=f"gdiff{t}")
                nc.gpsimd.tensor_tensor(out=gdiff[:], in0=gs[1][:],
                                        in1=gs[0][:],
                                        op=mybir.AluOpType.subtract)
                return gdiff

            def fixup_finish(t, cand, gdiff):
                """one exact difference-dot decides top-1; gather y row.

                s1 - s0 = (csq1-csq0) - 2*x.(c1-c0) = gdiff[D] - 2*dd
                (lower score wins; flip to cand1 iff s1 < s0).
                NB: tensor_tensor_reduce hard-faults TRN2; stt+accum_out
                is the working idiom for the dot."""
                tmp = work.tile([128, D], F32, tag="rescore_tmp")
                dd = work.tile([128, 1], F32, tag="dd")
                nc.vector.scalar_tensor_tensor(
                    out=tmp[:], in0=xn_t[t][:], scalar=1.0,
                    in1=gdiff[:, 0:D],
                    op0=mybir.AluOpType.bypass,
                    op1=mybir.AluOpType.mult, accum_out=dd[:])
                sdiff = work.tile([128, 1], F32, tag="sdiff")
                nc.vector.scalar_tensor_tensor(
                    out=sdiff[:], in0=dd[:], scalar=-2.0,
                    in1=gdiff[:, D:D + 1],
                    op0=mybir.AluOpType.mult, op1=mybir.AluOpType.add)
                flip = work.tile([128, 1], I32, tag="flip")
                nc.vector.tensor_scalar(
                    out=flip[:], in0=sdiff[:], scalar1=0.0, scalar2=None,
                    op0=mybir.AluOpType.is_lt)
                idx32 = work.tile([128, 1], I32, tag="idx32")
                nc.vector.tensor_copy(idx32[:], cand[0][:])
                nc.vector.copy_predicated(idx32[:], flip[:], cand[1][:])

                y = ypool.tile([128, D], BF16, tag="y")
                nc.gpsimd.indirect_dma_start(
                    out=y[:], out_offset=None, in_=cbyb_d.ap(),
                    in_offset=bass.IndirectOffsetOnAxis(ap=idx32[:, :1], axis=0))
                return y

            def emit_output(t, y):
                """bf16 mix matmul -> ACT drain -> noise accum-DMA -> store."""
                tsl = slice(t * 128, (t + 1) * 128)
                pm = ps_m.tile([128, D], F32, tag="pm")
                nc.tensor.matmul(pm[:, 0:512], w[:, tsl], y[:, 0:512],
                                 start=True, stop=True)
                nc.tensor.matmul(pm[:, 512:D], w[:, tsl], y[:, 512:D],
                                 start=True, stop=True)
                ob = io.tile([128, D], F32, tag="out")
                nc.scalar.copy(out=ob[:], in_=pm[:])
                # add DP noise inline in the DMA (SWDGE accumulate)
                nc.gpsimd.dma_start(out=ob[:], in_=noise_d.ap()[tsl, :],
                                    accum_op=mybir.AluOpType.add)
                nc.sync.dma_start(out_d.ap()[tsl, :], ob[:])

            # software pipeline: fixup dots for tile t run one tile late
            # (their gathers have landed by then -> no DVE head-of-line
            # blocking) and mix(t) ~3 tiles late so the whole chain never
            # stalls the PE.
            PIPE_MIX = 2
            pending_fix, pending_mix = [], []

            def tile_done(t, cand):
                gdiff = fixup_issue(t, cand)
                pending_fix.append((t, cand, gdiff))
                if len(pending_fix) > 1:
                    tf, cf, gf = pending_fix.pop(0)
                    y = fixup_finish(tf, cf, gf)
                    pending_mix.append((tf, y))
                    if len(pending_mix) > PIPE_MIX:
                        emit_output(*pending_mix.pop(0))

            scores_of, vals_of, idxl_of, nvp_done = {}, {}, {}, {}
            for j in range(len(sched)):
                t, vp = sched[j]
                incr = (t >= TT - 2)
                last = (t == TT - 1)
                if t not in scores_of:
                    scores_of[t] = sc_pool.tile([128, V], F32, tag="scores", name=f"scores{t}")
                    vals_of[t] = (work.tile([128, 32], F32, tag="vals32", name=f"vals32_{t}")
                                  if incr else None)
                    idxl_of[t] = (work.tile([128, 32], U32, tag="idxl", name=f"idxl{t}")
                                  if last else None)
                    nvp_done[t] = 0
                scores = scores_of[t]
                emit_block(j, scores)
                if incr:
                    vsl = slice(vp * 1024, (vp + 1) * 1024)
                    ssl = slice(vp * 8, (vp + 1) * 8)
                    nc.vector.max(vals_of[t][:, ssl], scores[:, vsl])
                    if last:
                        nc.vector.max_index(idxl_of[t][:, ssl],
                                            vals_of[t][:, ssl], scores[:, vsl])
                nvp_done[t] += 1
                if nvp_done[t] == NVP:
                    cand = emit_cands(scores, vals_of[t], idxl_of[t])
                    tile_done(t, cand)
            while pending_fix:
                tf, cf, gf = pending_fix.pop(0)
                pending_mix.append((tf, fixup_finish(tf, cf, gf)))
            for item in pending_mix:
                emit_output(*item)

    nc.compile()
    return nc


def _prep_inputs(base_embeddings, codebook, rand_idx, noise):
    """Build the 8 per-core input maps (all host-side numpy)."""
    import ml_dtypes
    x = np.ascontiguousarray(base_embeddings, dtype=np.float32)
    cb = np.ascontiguousarray(codebook, dtype=np.float32)
    ridx = np.asarray(rand_idx)
    nz = np.asarray(noise, dtype=np.float32)

    csq = (cb * cb).sum(-1, dtype=np.float32)              # [V]
    cbe = np.zeros((V, DE), np.float32)
    cbe[:, :D] = cb
    cbe[:, D] = csq
    csqc = (csq - CSQ_CENTER).astype(np.float32)
    # fp32 broadcast bias for ACT prefill (tiles >= 1)
    csqbc = np.ascontiguousarray(
        np.broadcast_to(-csqc[None, :], (128, V)).astype(np.float32))
    # fp16 hi/lo pair for tile 0's in-matmul csq bias
    r1 = csqc.astype(np.float16)
    r2 = (csqc - r1.astype(np.float32)).astype(np.float16)
    csqR = np.ascontiguousarray(np.stack([r1, r2]))        # [2, V] fp16
    csqL = np.full((2, 128), -1.0, np.float16)
    cbyb = cb.astype(ml_dtypes.bfloat16)                   # [V, D] bf16

    # merge constants: [128, 64] f32; cols 0:32 iota, 32:64 1024*(c//8)
    mc = np.zeros((128, 64), np.float32)
    mc[:, 0:32] = np.arange(32, dtype=np.float32)[None, :]
    mc[:, 32:64] = (1024.0 * (np.arange(32) // 8)).astype(np.float32)[None, :]
    mc = np.ascontiguousarray(mc)

    # pre-tile [D, V] -> [128, (v, k, 512)] v-block-major layout
    cbT = cb.T.reshape(KC, 128, NV, 512).transpose(1, 2, 0, 3).reshape(128, KC * V)
    cbT = np.ascontiguousarray(cbT).astype(ml_dtypes.bfloat16)

    shared = {"cbe": cbe, "csqbc": csqbc, "csqL": csqL, "csqR": csqR,
              "cbyb": cbyb, "mc": mc, "cbT": cbT}

    in_maps = []
    for c in range(N_CORES):
        ssl = slice(c * SS, (c + 1) * SS)
        # tokens t = s_local*16 + b
        xc = x[:, ssl, :].transpose(1, 0, 2).reshape(T, D)
        xT2 = (2.0 * xc).T                                 # [D, T] fp32
        # pre-tile [D, T] -> [128, (t, k, 128)] tile-major layout
        xT2 = np.ascontiguousarray(
            xT2.reshape(KC, 128, TT, 128).transpose(1, 2, 0, 3).reshape(128, KC * T))
        nzc = np.ascontiguousarray(
            DP_EPSILON * nz[:, ssl, :].transpose(1, 0, 2).reshape(T, D))
        rc = ridx[:, ssl, :]                               # [B, SS, K]
        wm = np.zeros((TT, 128, 128), np.float32)
        for tt in range(TT):
            for g in range(8):
                s_local = tt * 8 + g
                r = rc[:, s_local, :]                      # [B, K] in [0,B)
                cnt = np.zeros((B, B), np.float32)         # [dst=b, src]
                for bdst in range(B):
                    np.add.at(cnt[bdst], r[bdst], 1.0)
                wm[tt, g * 16:(g + 1) * 16, g * 16:(g + 1) * 16] = cnt.T / K
        wm_t = np.ascontiguousarray(
            wm.transpose(1, 0, 2).reshape(128, TT * 128)).astype(ml_dtypes.bfloat16)
        m = {"w": wm_t, "noise": nzc, **shared,
             "xT": xT2.astype(ml_dtypes.bfloat16),
             "xn": np.ascontiguousarray(
                 xc.reshape(TT, 128, D).transpose(1, 0, 2).reshape(128, TT * D))}
        in_maps.append(m)
    return in_maps


def kernel(base_embeddings, codebook, rand_idx, noise, _results_out=None):
    if "nc" not in _CACHED:
        _CACHED["nc"] = _build_nc()
    nc = _CACHED["nc"]
    in_maps = _prep_inputs(base_embeddings, codebook, rand_idx, noise)
    res = run_bass_kernel_spmd(nc, in_maps, list(range(N_CORES)))
    if _results_out is not None:
        _results_out.append(res)
    outs = []
    for c in range(N_CORES):
        oc = res.results[c]["out"].reshape(SS, B, D).transpose(1, 0, 2)
        outs.append(oc)
    return np.ascontiguousarray(np.concatenate(outs, axis=1))
